# revision 1
# baseline (speedup 1.0000x reference)
"""Trainium2 Bass kernel for nn_Classifier (segment_reduce).

Computation (reference semantics):
  attn  = concat(emb, pos) @ W_attn + b_attn          (S, T, 1)
  w     = softmax(attn, axis=1)                        per-segment over T
  segv  = sum_t w * emb                                (S, BERT)
  vecs  = segment_sum(segv, segment_ids, 64)           (64, BERT)
  out   = sigmoid(lrelu(lrelu(vecs@W1+b1)@W2+b2)@W3+b3)

Sharding: data-parallel over S across 8 NeuronCores (32 segments each),
AllReduce of the (64, 768) comment partials, replicated MLP.

Notes:
 - b_attn shifts all logits of a segment equally -> softmax-invariant -> dropped.
 - softmax normalization (1/sum_t exp) is folded into the host-built
   one-hot segment->comment matrix, scaled on device by 1/den per segment.
 - exp() without max-subtraction: logits ~ N(0, 0.6^2), safe in fp32.
"""

import sys

sys.path.insert(0, "/opt/trn_rl_repo")

import numpy as np

# Full-problem dims (hardcoded per contract)
S, T, BERT, POS = 256, 512, 768, 128
FEAT = BERT + POS
H1 = 1024
NCLS = 6
NCOM = 64
NCORES = 8

_CACHE = {}


class _ProbeDone(Exception):
    pass


def _probe_out(nc, out_d, ep, L_sb, E_sb, partial_sb, vecs, y, level, ncom, ncls):
    if level <= 0:
        src = ep[0:ncom, 0, 0:ncls]
    elif level == 1:
        src = L_sb[0:ncom, 0:ncls]
    elif level == 2:
        src = svT_sb[0:ncom, 0:ncls]  # (E_sb passed here)
    elif level == 3:
        src = partial_sb[:, 0:ncls]
    else:
        src = vecs[:, 0:ncls]
    nc.sync.dma_start(out_d, src)


def build_nc(n_cores, sl, t, bert, pos, h1, ncls, ncom, level=5):
    """Build the SPMD Bass program for one core (sl segments/core)."""
    import concourse.bass as bass
    import concourse.mybir as mybir
    import concourse.tile as tile
    from concourse import bacc
    from concourse.masks import make_identity

    f32 = mybir.dt.float32
    AF = mybir.ActivationFunctionType
    OP = mybir.AluOpType
    AX = mybir.AxisListType

    feat = bert + pos
    nt = t // 128          # token tiles per segment
    nfb = bert // 128      # feature blocks of emb
    nk1 = bert // 128      # k tiles layer1
    nk2 = h1 // 128        # k tiles layer2/3

    nc = bacc.Bacc(
        "TRN2", target_bir_lowering=False, debug=False, num_devices=n_cores
    )

    emb_d = nc.dram_tensor("emb", [sl, t, bert], f32, kind="ExternalInput").ap()
    pos_d = nc.dram_tensor("pos", [sl, t, pos], f32, kind="ExternalInput").ap()
    wab_d = nc.dram_tensor("wab", [128, feat], f32, kind="ExternalInput").ap()
    oneh_d = nc.dram_tensor("oneh", [sl, ncom], f32, kind="ExternalInput").ap()
    w1_d = nc.dram_tensor("w1", [bert, h1], f32, kind="ExternalInput").ap()
    b1_d = nc.dram_tensor("b1", [1, h1], f32, kind="ExternalInput").ap()
    w2_d = nc.dram_tensor("w2", [h1, h1], f32, kind="ExternalInput").ap()
    b2_d = nc.dram_tensor("b2", [1, h1], f32, kind="ExternalInput").ap()
    w3_d = nc.dram_tensor("w3", [h1, ncls], f32, kind="ExternalInput").ap()
    b3_d = nc.dram_tensor("b3", [1, ncls], f32, kind="ExternalInput").ap()
    out_d = nc.dram_tensor("out", [ncom, ncls], f32, kind="ExternalOutput").ap()

    with tile.TileContext(nc) as tc:
        with (
            tc.tile_pool(name="const", bufs=1) as const_pool,
            tc.tile_pool(name="ep", bufs=4) as ep_pool,
            tc.tile_pool(name="work", bufs=1) as work,
            tc.tile_pool(name="psv", bufs=2, space="PSUM") as psv,
            tc.tile_pool(name="pmisc", bufs=2, space="PSUM") as pmisc,
            tc.tile_pool(name="dram", bufs=1, space="DRAM") as dram,
        ):
            def _emit():
                # ---- constants ----
                wab_sb = const_pool.tile([128, feat], f32)
                nc.sync.dma_start(wab_sb, wab_d)
                oneh_sb = const_pool.tile([sl, ncom], f32)
                nc.sync.dma_start(oneh_sb, oneh_d)
                identity = const_pool.tile([128, 128], f32)
                make_identity(nc, identity)
                ones_sb = const_pool.tile([128, 64], f32)
                nc.gpsimd.memset(ones_sb, 1.0)

                # ---- persistent working tiles ----
                L_sb = work.tile([128, nt * sl], f32)     # logits, col = s*nt + i
                E_sb = work.tile([128, nt * sl], f32)     # exp(logits)
                segvecs = work.tile([sl, bert], f32)      # unnormalized segvecs
                dummy = work.tile([128, 1], f32)          # discard for fused reduce

                # ---- main loop over local segments ----
                for s in range(sl):
                    ep = ep_pool.tile([128, nt, feat], f32, tag="ep")
                    nc.sync.dma_start(
                        ep[:, :, 0:bert],
                        emb_d[s].rearrange("(i p) f -> p i f", p=128),
                    )
                    nc.sync.dma_start(
                        ep[:, :, bert:feat],
                        pos_d[s].rearrange("(i p) f -> p i f", p=128),
                    )
                    # attention logits: fused multiply + free-dim reduce
                    for i in range(nt if level >= 1 else 0):
                        nc.vector.scalar_tensor_tensor(
                            dummy.broadcast_to([128, feat]),
                            ep[:, i, :],
                            1.0,
                            wab_sb,
                            op0=OP.mult,
                            op1=OP.mult,
                            accum_out=L_sb[:, nt * s + i : nt * s + i + 1],
                        )
                    # e = exp(logits)
                    if level < 2:
                        continue
                    nc.scalar.activation(
                        E_sb[:, nt * s : nt * s + nt],
                        L_sb[:, nt * s : nt * s + nt],
                        AF.Exp,
                    )
                    # pooling: segvec[s] = emb^T-weighted sum over tokens.
                    # e column is the (128,1) stationary; wide emb slices are
                    # the moving operand -> (1, bert) PSUM row per segment.
                    sv = psv.tile([1, bert], f32, tag="sv")
                    for i in range(nt):
                        col = nt * s + i
                        for n0 in range(0, bert, 512):
                            n1 = min(n0 + 512, bert)
                            nc.tensor.matmul(
                                sv[0:1, n0:n1],
                                E_sb[:, col : col + 1],
                                ep[:, i, n0:n1],
                                start=(i == 0),
                                stop=(i == nt - 1),
                            )
                    stage = work.tile([1, bert], f32, tag="stage", bufs=3, name="stage")
                    nc.scalar.copy(stage, sv)
                    # partition-scatter into row s of segvecs
                    nc.sync.dma_start(segvecs[s : s + 1, :], stage)

                # ---- MLP weights: loaded only now so the 7MB doesn't
                # delay the first emb tiles at kernel start ----
                b1_sb = const_pool.tile([1, h1], f32)
                nc.sync.dma_start(b1_sb, b1_d)
                b2_sb = const_pool.tile([1, h1], f32)
                nc.sync.dma_start(b2_sb, b2_d)
                b3_sb = const_pool.tile([1, ncls], f32)
                nc.sync.dma_start(b3_sb, b3_d)
                w1_sb = const_pool.tile([128, nk1, h1], f32)
                nc.sync.dma_start(w1_sb, w1_d.rearrange("(j p) h -> p j h", p=128))
                w2_sb = const_pool.tile([128, nk2, h1], f32)
                nc.sync.dma_start(w2_sb, w2_d.rearrange("(j p) h -> p j h", p=128))
                w3_sb = const_pool.tile([128, nk2, ncls], f32)
                nc.sync.dma_start(w3_sb, w3_d.rearrange("(j p) h -> p j h", p=128))

                # ---- denominators: den[s] = sum_t e ----
                if level < 3:
                    _probe_out(nc, out_d, ep, L_sb, E_sb, None, None, None, level, ncom, ncls)
                    return
                den_row = pmisc.tile([1, nt * sl], f32, tag="m")
                nc.tensor.matmul(
                    den_row, ones_sb[:, 0:1], E_sb, start=True, stop=True
                )
                den_sb = work.tile([1, sl], f32)
                nc.vector.tensor_reduce(
                    den_sb,
                    den_row.rearrange("p (s i) -> p s i", i=nt),
                    axis=AX.X,
                    op=OP.add,
                )
                den_col = pmisc.tile([sl, 1], f32, tag="m")
                nc.tensor.matmul(
                    den_col, den_sb, ones_sb[0:1, 0:1], start=True, stop=True
                )
                inv_den = work.tile([sl, 1], f32)
                nc.vector.reciprocal(inv_den, den_col)
                oneh_sc = work.tile([sl, ncom], f32)
                nc.vector.tensor_scalar_mul(oneh_sc, oneh_sb, inv_den)

                # ---- comment partials: (ncom, bert) = oneh_sc^T @ segvecs ----
                cm = pmisc.tile([ncom, bert], f32, tag="m")
                for n0 in range(0, bert, 512):
                    n1 = min(n0 + 512, bert)
                    nc.tensor.matmul(
                        cm[:, n0:n1],
                        oneh_sc,
                        segvecs[:, n0:n1],
                        start=True,
                        stop=True,
                    )
                partial_sb = work.tile([ncom, bert], f32)
                nc.vector.tensor_copy(partial_sb, cm)

                if level < 4:
                    _probe_out(nc, out_d, ep, L_sb, E_sb, partial_sb, None, None, level, ncom, ncls)
                    return
                # ---- AllReduce over cores ----
                # NOTE: addr_space="Shared" collective outputs crash this runtime
                # (NRT_EXEC_UNIT_UNRECOVERABLE); Local works and is plenty fast
                # for a 196KB all-reduce.
                ar_in = dram.tile([ncom, bert], f32)
                ar_out = dram.tile([ncom, bert], f32)
                nc.sync.dma_start(ar_in, partial_sb)
                nc.gpsimd.collective_compute(
                    "AllReduce",
                    OP.add,
                    replica_groups=[list(range(n_cores))],
                    ins=[ar_in.opt()],
                    outs=[ar_out.opt()],
                )
                vecs = work.tile([ncom, bert], f32)
                nc.sync.dma_start(vecs, ar_out)

                if level < 5:
                    _probe_out(nc, out_d, ep, L_sb, E_sb, partial_sb, vecs, None, level, ncom, ncls)
                    return
                # ---- MLP (replicated on every core) ----
                def linear(x_sb, kdim, ndim, w_sb, b_sb, act):
                    nk = kdim // 128
                    xT = work.tile([128, nk, ncom], f32, tag="xT", name="xT")
                    for j in range(nk):
                        tp2 = pmisc.tile([128, ncom], f32, tag="m", name="tp2")
                        nc.tensor.transpose(
                            tp2,
                            x_sb[:, 128 * j : 128 * (j + 1)],
                            identity[0:ncom, 0:ncom],
                        )
                        nc.scalar.copy(xT[:, j, :], tp2)
                    h_ps = pmisc.tile([ncom, ndim], f32, tag="m", name="h_ps")
                    for n0 in range(0, ndim, 512):
                        n1 = min(n0 + 512, ndim)
                        for j in range(nk):
                            nc.tensor.matmul(
                                h_ps[:, n0:n1],
                                xT[:, j, :],
                                w_sb[:, j, n0:n1],
                                start=(j == 0),
                                stop=False,
                            )
                        nc.tensor.matmul(
                            h_ps[:, n0:n1],
                            ones_sb[0:1, 0:ncom],
                            b_sb[:, n0:n1],
                            start=False,
                            stop=True,
                        )
                    y_sb = work.tile([ncom, ndim], f32, tag="y", name="y_sb")
                    if act == "lrelu":
                        # y = max(0.01*x, x); DVE can read PSUM on at most one
                        # port, so stage x into SBUF first.
                        x_sb = work.tile([ncom, ndim], f32, tag="xs", name="x_sb")
                        nc.scalar.copy(x_sb, h_ps)
                        nc.vector.scalar_tensor_tensor(
                            y_sb, x_sb, 0.01, x_sb, op0=OP.mult, op1=OP.max
                        )
                    else:
                        nc.scalar.activation(y_sb, h_ps, AF.Sigmoid)
                    return y_sb

                h1_sb = linear(vecs, bert, h1, w1_sb, b1_sb, "lrelu")
                h2_sb = linear(h1_sb, h1, h1, w2_sb, b2_sb, "lrelu")
                y_sb = linear(h2_sb, h1, ncls, w3_sb, b3_sb, "sigmoid")

                nc.sync.dma_start(out_d, y_sb)

            _emit()

    nc.compile()
    return nc


def make_in_maps(
    embeddings,
    position_encodings,
    W_attn,
    W1,
    b1,
    W2,
    b2,
    W3,
    b3,
    segment_ids,
    n_cores,
    ncom,
):
    """Host-side sharding: slice S across cores, build per-core one-hot."""
    f32 = np.float32
    s_total = embeddings.shape[0]
    sl = s_total // n_cores
    feat = embeddings.shape[2] + position_encodings.shape[2]

    wa = np.asarray(W_attn, dtype=f32).reshape(-1)
    wab = np.ascontiguousarray(np.tile(wa[None, :], (128, 1)))
    assert wab.shape == (128, feat)

    seg = np.asarray(segment_ids).astype(np.int64).reshape(-1)
    common = {
        "wab": wab,
        "w1": np.ascontiguousarray(np.asarray(W1, dtype=f32)),
        "b1": np.ascontiguousarray(np.asarray(b1, dtype=f32).reshape(1, -1)),
        "w2": np.ascontiguousarray(np.asarray(W2, dtype=f32)),
        "b2": np.ascontiguousarray(np.asarray(b2, dtype=f32).reshape(1, -1)),
        "w3": np.ascontiguousarray(np.asarray(W3, dtype=f32)),
        "b3": np.ascontiguousarray(np.asarray(b3, dtype=f32).reshape(1, -1)),
    }
    in_maps = []
    for c in range(n_cores):
        oneh = np.zeros((sl, ncom), dtype=f32)
        local = seg[c * sl : (c + 1) * sl]
        oneh[np.arange(sl), local] = 1.0
        in_maps.append(
            {
                "emb": np.ascontiguousarray(
                    embeddings[c * sl : (c + 1) * sl], dtype=f32
                ),
                "pos": np.ascontiguousarray(
                    position_encodings[c * sl : (c + 1) * sl], dtype=f32
                ),
                "oneh": oneh,
                **common,
            }
        )
    return in_maps


def kernel(
    embeddings,
    position_encodings,
    W_attn,
    b_attn,
    W1,
    b1,
    W2,
    b2,
    W3,
    b3,
    segment_ids,
    num_comments,
):
    from concourse.bass_utils import run_bass_kernel_spmd

    assert int(num_comments) == NCOM
    assert embeddings.shape == (S, T, BERT)
    assert position_encodings.shape == (S, T, POS)
    # b_attn shifts every logit of a segment equally -> softmax-invariant.

    key = "full"
    if key not in _CACHE:
        _CACHE[key] = build_nc(NCORES, S // NCORES, T, BERT, POS, H1, NCLS, NCOM)
    nc = _CACHE[key]

    in_maps = make_in_maps(
        embeddings,
        position_encodings,
        W_attn,
        W1,
        b1,
        W2,
        b2,
        W3,
        b3,
        segment_ids,
        NCORES,
        NCOM,
    )
    res = run_bass_kernel_spmd(nc, in_maps, list(range(NCORES)))
    return np.asarray(res.results[0]["out"], dtype=np.float32)



# revision 16
# speedup vs baseline: 1.6218x; 1.6218x over previous
"""Trainium2 Bass kernel for nn_Classifier (segment_reduce).

Computation (reference semantics):
  attn  = concat(emb, pos) @ W_attn + b_attn          (S, T, 1)
  w     = softmax(attn, axis=1)                        per-segment over T
  segv  = sum_t w * emb                                (S, BERT)
  vecs  = segment_sum(segv, segment_ids, 64)           (64, BERT)
  out   = sigmoid(lrelu(lrelu(vecs@W1+b1)@W2+b2)@W3+b3)

Sharding: data-parallel over S across 8 NeuronCores (32 segments each),
AllReduce of the (768, 64) transposed comment partials, replicated MLP.

Key optimizations vs the fp32 baseline:
 - emb||pos concatenated, token-permuted to (p, i) order (softmax pooling
   is token-permutation invariant) and cast to bf16 ON HOST: halves HBM
   traffic, gives 7KB contiguous partition lines, and runs the PE at
   1 cycle/row instead of fp32's LOW_HIGH 2-pass mode.
 - AllReduce split into two chunks: segments 0:16 reduce + AllReduce
   launch mid-loop (hidden under the second half of the segment loop);
   only the second 196KB AllReduce is exposed at the end.
 - comment partials computed TRANSPOSED (feature-major), so the MLP runs
   transpose-free: h^T tiles (128 feat x 64 comments), per-partition
   bias fused into the PSUM->SBUF copy, lrelu as max(0.01x, x) on DVE.
 - dependent DMAs (collective staging, segvec scatter) issue from
   scalar/gpsimd queues so they never head-of-line block the streaming
   ep DMAs on the sync queue.

Notes:
 - b_attn shifts all logits of a segment equally -> softmax-invariant -> dropped.
 - softmax normalization (1/sum_t exp) is folded into the host-built
   one-hot segment->comment matrix, scaled on device by 1/den per segment.
 - exp() without max-subtraction: logits ~ N(0, 0.6^2), safe.
 - addr_space="Shared" collective outputs crash this runtime
   (NRT_EXEC_UNIT_UNRECOVERABLE); Local DRAM tiles work.
"""

import sys

sys.path.insert(0, "/opt/trn_rl_repo")

import numpy as np

# Full-problem dims (hardcoded per contract)
S, T, BERT, POS = 256, 512, 768, 128
FEAT = BERT + POS
H1 = 1024
NCLS = 6
NCOM = 64
NCORES = 8
NT = T // 128  # token tiles per segment
NB = BERT // 128  # feature blocks of emb (6)
NJ1 = BERT // 128  # k tiles layer1 (6)
NJ2 = H1 // 128  # k tiles layer2/3 (8)

_CACHE = {}


def build_nc(n_cores, sl):
    """Build the SPMD Bass program for one core (sl segments/core)."""
    import concourse.bass as bass
    import concourse.mybir as mybir
    import concourse.tile as tile
    from concourse import bacc

    f32 = mybir.dt.float32
    bf16 = mybir.dt.bfloat16
    AF = mybir.ActivationFunctionType
    OP = mybir.AluOpType
    AX = mybir.AxisListType

    half = sl // 2  # chunk boundary for the split AllReduce

    nc = bacc.Bacc(
        "TRN2", target_bir_lowering=False, debug=False, num_devices=n_cores
    )

    ep_d = nc.dram_tensor("ep", [sl, 128, NT * FEAT], bf16, kind="ExternalInput").ap()
    wab_d = nc.dram_tensor("wab", [128, FEAT], bf16, kind="ExternalInput").ap()
    oneh_d = nc.dram_tensor("oneh", [sl, NCOM], f32, kind="ExternalInput").ap()
    w1_d = nc.dram_tensor("w1", [128, NJ1, H1], bf16, kind="ExternalInput").ap()
    w2_d = nc.dram_tensor("w2", [128, NJ2, H1], bf16, kind="ExternalInput").ap()
    w3_d = nc.dram_tensor("w3", [128, NJ2, NCLS], bf16, kind="ExternalInput").ap()
    b1t_d = nc.dram_tensor("b1t", [128, NJ2], f32, kind="ExternalInput").ap()
    b2t_d = nc.dram_tensor("b2t", [128, NJ2], f32, kind="ExternalInput").ap()
    b3_d = nc.dram_tensor("b3", [1, NCLS], f32, kind="ExternalInput").ap()
    out_d = nc.dram_tensor("out", [NCOM, NCLS], f32, kind="ExternalOutput").ap()

    with tile.TileContext(nc) as tc:
        with (
            tc.tile_pool(name="const", bufs=1) as const_pool,
            tc.tile_pool(name="ep", bufs=10) as ep_pool,
            tc.tile_pool(name="work", bufs=1) as work,
            tc.tile_pool(name="psv", bufs=2, space="PSUM") as psv,
            tc.tile_pool(name="pcm", bufs=2, space="PSUM") as pcm,
            tc.tile_pool(name="pmlp", bufs=2, space="PSUM") as pmlp,
            tc.tile_pool(name="dram", bufs=1, space="DRAM") as dram,
        ):
            # ---- constants (small, loaded first) ----
            wab_sb = const_pool.tile([128, FEAT], bf16)
            nc.sync.dma_start(wab_sb, wab_d)
            # one-hot loaded as two half tiles so every engine read starts
            # at partition 0 (partition base must be 0/32/64)
            oneh_sbA = const_pool.tile([sl // 2, NCOM], f32)
            nc.sync.dma_start(oneh_sbA, oneh_d[0 : sl // 2])
            oneh_sbB = const_pool.tile([sl // 2, NCOM], f32)
            nc.sync.dma_start(oneh_sbB, oneh_d[sl // 2 : sl])
            ones_bf = const_pool.tile([128, 1], bf16)
            nc.gpsimd.memset(ones_bf, 1.0)
            onesf = const_pool.tile([1, NCOM], f32)
            nc.gpsimd.memset(onesf, 1.0)

            # ---- persistent working tiles ----
            L_sb = work.tile([128, NT * sl], f32)   # logits, col = 4*s + i
            E_sb = work.tile([128, NT * sl], bf16)  # exp(logits)
            # unnormalized segvecs, one tile per AllReduce chunk so each
            # matmul stationary read starts at partition 0
            segvecsA = work.tile([sl // 2, BERT], f32)
            segvecsB = work.tile([sl // 2, BERT], f32)
            dummy = work.tile([128, 1], bf16)       # discard for fused reduce

            def chunk_reduce(lo, hi, segv, oneh_half, tag):
                """den + scaled one-hot + transposed comment partial matmul +
                AllReduce for local segments [lo, hi). Returns the SBUF tile
                that will hold the reduced (128, NB*NCOM) partial."""
                n = hi - lo
                den_ps = pcm.tile([1, NT * n], f32, tag="cm")
                nc.tensor.matmul(
                    den_ps, ones_bf, E_sb[:, NT * lo : NT * hi], start=True, stop=True
                )
                den_sb = work.tile([1, n], f32, tag=f"den{tag}", name=f"den{tag}")
                nc.vector.tensor_reduce(
                    den_sb,
                    den_ps.rearrange("p (s i) -> p s i", i=NT),
                    axis=AX.X,
                    op=OP.add,
                )
                denT = pcm.tile([n, 1], f32, tag="cm")
                nc.tensor.matmul(denT, den_sb, onesf[0:1, 0:1], start=True, stop=True)
                inv_den = work.tile([n, 1], f32, tag=f"inv{tag}", name=f"inv{tag}")
                nc.vector.reciprocal(inv_den, denT)
                oneh_sc = work.tile([n, NCOM], f32, tag=f"ohs{tag}", name=f"ohs{tag}")
                nc.vector.tensor_scalar_mul(oneh_sc, oneh_half, inv_den)
                # cmT[b*128+p, c] = sum_s segvecs[s, b*128+p] * oneh_sc[s, c]
                cmT_ps = pcm.tile([128, NB * NCOM], f32, tag="cm")
                for b in range(NB):
                    nc.tensor.matmul(
                        cmT_ps[:, b * NCOM : (b + 1) * NCOM],
                        segv[0:n, 128 * b : 128 * (b + 1)],
                        oneh_sc,
                        start=True,
                        stop=True,
                    )
                cmT_sb = work.tile(
                    [128, NB * NCOM], f32, tag=f"cmT{tag}", name=f"cmT{tag}"
                )
                nc.vector.tensor_copy(cmT_sb, cmT_ps)
                ar_in = dram.tile([128, NB * NCOM], f32, tag=f"ari{tag}")
                ar_out = dram.tile([128, NB * NCOM], f32, tag=f"aro{tag}")
                nc.gpsimd.dma_start(ar_in, cmT_sb)
                nc.gpsimd.collective_compute(
                    "AllReduce",
                    OP.add,
                    replica_groups=[list(range(n_cores))],
                    ins=[ar_in.opt()],
                    outs=[ar_out.opt()],
                )
                vecs_sb = work.tile(
                    [128, NB * NCOM], f32, tag=f"vec{tag}", name=f"vec{tag}"
                )
                nc.gpsimd.dma_start(vecs_sb, ar_out)
                return vecs_sb

            # ---- main loop over local segments ----
            vecsA = None
            sv = None
            for s in range(sl):
                ep = ep_pool.tile([128, NT * FEAT], bf16, tag="ep")
                # Split each segment's load across DMA engines; the first
                # few segments split 4-ways to cut pipeline-fill latency.
                nd = 4 if s < 4 else 2
                step = (NT * FEAT) // nd
                for k in range(nd):
                    nc.sync.dma_start(
                        ep[:, k * step : (k + 1) * step],
                        ep_d[s, :, k * step : (k + 1) * step],
                    )
                # attention logits: fused multiply + free-dim reduce
                for i in range(NT):
                    col = NT * s + i
                    nc.vector.scalar_tensor_tensor(
                        dummy.broadcast_to([128, FEAT]),
                        ep[:, i * FEAT : (i + 1) * FEAT],
                        1.0,
                        wab_sb,
                        op0=OP.mult,
                        op1=OP.mult,
                        accum_out=L_sb[:, col : col + 1],
                    )
                # e = exp(logits)
                nc.scalar.activation(
                    E_sb[:, NT * s : NT * s + NT],
                    L_sb[:, NT * s : NT * s + NT],
                    AF.Exp,
                )
                # pooling: segvec[s] = sum_t e[t] * emb[t, :]
                sv = psv.tile([1, BERT], f32, tag="sv")
                for i in range(NT):
                    col = NT * s + i
                    for n0, n1 in ((0, 512), (512, BERT)):
                        nc.tensor.matmul(
                            sv[0:1, n0:n1],
                            E_sb[:, col : col + 1],
                            ep[:, i * FEAT + n0 : i * FEAT + n1],
                            start=(i == 0),
                            stop=(i == NT - 1),
                        )
                stage = work.tile([1, BERT], f32, tag="stage", bufs=4, name="stage")
                nc.scalar.copy(stage, sv)
                # partition-scatter into row s of the half's segvec tile;
                # issued from the scalar queue right after its producing copy.
                segv = segvecsA if s < half else segvecsB
                row = s % half
                nc.scalar.dma_start(segv[row : row + 1, :], stage)

                if s == 2:
                    # MLP weights: emitted here so the sync queue issues
                    # them early; ~3.7MB bf16 trickles in under the loop.
                    b1t_sb = const_pool.tile([128, NJ2], f32)
                    nc.sync.dma_start(b1t_sb, b1t_d)
                    b2t_sb = const_pool.tile([128, NJ2], f32)
                    nc.sync.dma_start(b2t_sb, b2t_d)
                    b3_sb = const_pool.tile([1, NCLS], f32)
                    nc.sync.dma_start(b3_sb, b3_d)
                    w1_sb = const_pool.tile([128, NJ1, H1], bf16)
                    nc.sync.dma_start(w1_sb, w1_d)
                    w2_sb = const_pool.tile([128, NJ2, H1], bf16)
                    nc.sync.dma_start(w2_sb, w2_d)
                    w3_sb = const_pool.tile([128, NJ2, NCLS], bf16)
                    nc.sync.dma_start(w3_sb, w3_d)

                if s == half - 1:
                    # first-chunk comment partials + AllReduce, hidden
                    # under the second half of the segment loop
                    vecsA = chunk_reduce(0, half, segvecsA, oneh_sbA, "A")

            vecsB = chunk_reduce(half, sl, segvecsB, oneh_sbB, "B")

            # vecsT = vecsA + vecsB, cast to bf16 for the MLP
            vecsT = work.tile([128, NB * NCOM], bf16)
            nc.vector.scalar_tensor_tensor(
                vecsT, vecsA, 1.0, vecsB, op0=OP.mult, op1=OP.add
            )

            # ---- MLP, transpose-free (feature-major activations) ----
            def layer_T(x_bf, nj, nout, w_sb, bT_sb, lid):
                """x_bf: (128, nj*NCOM) bf16 -> returns (128, nout*NCOM) bf16
                with hT[n*128+p, c] = lrelu(sum_jp W[jp, n*128+p] x[jp, c] + b)."""
                y = work.tile([128, nout * NCOM], bf16, tag=f"y{lid}", name="yT")
                for n in range(nout):
                    hps = pmlp.tile([128, NCOM], f32, tag="h")
                    for j in range(nj):
                        nc.tensor.matmul(
                            hps,
                            w_sb[:, j, 128 * n : 128 * (n + 1)],
                            x_bf[:, j * NCOM : (j + 1) * NCOM],
                            start=(j == 0),
                            stop=(j == nj - 1),
                        )
                    xs = work.tile([128, NCOM], f32, tag=f"xs{lid}", bufs=3, name="xs")
                    # PSUM->SBUF copy with fused per-partition bias add
                    nc.scalar.activation(
                        xs, hps, AF.Identity, bias=bT_sb[:, n : n + 1]
                    )
                    # lrelu: y = max(0.01*x, x)
                    nc.vector.scalar_tensor_tensor(
                        y[:, n * NCOM : (n + 1) * NCOM],
                        xs,
                        0.01,
                        xs,
                        op0=OP.mult,
                        op1=OP.max,
                    )
                return y

            h1T = layer_T(vecsT, NJ1, NJ2, w1_sb, b1t_sb, 1)
            h2T = layer_T(h1T, NJ2, NJ2, w2_sb, b2t_sb, 2)

            # layer 3: out (NCOM, NCLS) = sum_j h2T_j^T @ W3_j + b3
            ops = pmlp.tile([NCOM, NCLS], f32, tag="h")
            for j in range(NJ2):
                nc.tensor.matmul(
                    ops,
                    h2T[:, j * NCOM : (j + 1) * NCOM],
                    w3_sb[:, j, :],
                    start=(j == 0),
                    stop=False,
                )
            nc.tensor.matmul(ops, onesf, b3_sb, start=False, stop=True)
            out_sb = work.tile([NCOM, NCLS], f32)
            nc.scalar.activation(out_sb, ops, AF.Sigmoid)
            nc.sync.dma_start(out_d, out_sb)

    nc.compile()
    return nc


def make_in_maps(
    embeddings,
    position_encodings,
    W_attn,
    W1,
    b1,
    W2,
    b2,
    W3,
    b3,
    segment_ids,
    n_cores,
    ncom,
):
    """Host-side sharding: slice S across cores, build per-core one-hot,
    concat+permute+bf16-cast the token stream, transpose weights."""
    import ml_dtypes

    f32 = np.float32
    bf16 = ml_dtypes.bfloat16
    s_total = embeddings.shape[0]
    sl = s_total // n_cores

    emb = np.asarray(embeddings, dtype=f32)
    pos = np.asarray(position_encodings, dtype=f32)
    epc = np.concatenate([emb, pos], axis=2)  # (S, T, FEAT)
    # token t = i*128 + p  ->  (p, i); softmax pooling is token-perm invariant
    epc = epc.reshape(s_total, NT, 128, FEAT).transpose(0, 2, 1, 3)
    epc = np.ascontiguousarray(epc).reshape(s_total, 128, NT * FEAT).astype(bf16)

    wa = np.asarray(W_attn, dtype=f32).reshape(-1)
    wab = np.ascontiguousarray(np.tile(wa[None, :], (128, 1))).astype(bf16)

    w1r = np.ascontiguousarray(
        np.asarray(W1, dtype=f32).reshape(NJ1, 128, H1).transpose(1, 0, 2)
    ).astype(bf16)
    w2r = np.ascontiguousarray(
        np.asarray(W2, dtype=f32).reshape(NJ2, 128, H1).transpose(1, 0, 2)
    ).astype(bf16)
    w3r = np.ascontiguousarray(
        np.asarray(W3, dtype=f32).reshape(NJ2, 128, NCLS).transpose(1, 0, 2)
    ).astype(bf16)
    b1t = np.ascontiguousarray(np.asarray(b1, dtype=f32).reshape(NJ2, 128).T)
    b2t = np.ascontiguousarray(np.asarray(b2, dtype=f32).reshape(NJ2, 128).T)
    b3r = np.ascontiguousarray(np.asarray(b3, dtype=f32).reshape(1, -1))

    seg = np.asarray(segment_ids).astype(np.int64).reshape(-1)
    common = {
        "wab": wab,
        "w1": w1r,
        "w2": w2r,
        "w3": w3r,
        "b1t": b1t,
        "b2t": b2t,
        "b3": b3r,
    }
    in_maps = []
    for c in range(n_cores):
        oneh = np.zeros((sl, ncom), dtype=f32)
        local = seg[c * sl : (c + 1) * sl]
        oneh[np.arange(sl), local] = 1.0
        in_maps.append(
            {
                "ep": np.ascontiguousarray(epc[c * sl : (c + 1) * sl]),
                "oneh": oneh,
                **common,
            }
        )
    return in_maps


def kernel(
    embeddings,
    position_encodings,
    W_attn,
    b_attn,
    W1,
    b1,
    W2,
    b2,
    W3,
    b3,
    segment_ids,
    num_comments,
):
    from concourse.bass_utils import run_bass_kernel_spmd

    assert int(num_comments) == NCOM
    assert embeddings.shape == (S, T, BERT)
    assert position_encodings.shape == (S, T, POS)
    # b_attn shifts every logit of a segment equally -> softmax-invariant.

    key = "full"
    if key not in _CACHE:
        _CACHE[key] = build_nc(NCORES, S // NCORES)
    nc = _CACHE[key]

    in_maps = make_in_maps(
        embeddings,
        position_encodings,
        W_attn,
        W1,
        b1,
        W2,
        b2,
        W3,
        b3,
        segment_ids,
        NCORES,
        NCOM,
    )
    res = run_bass_kernel_spmd(nc, in_maps, list(range(NCORES)))
    return np.asarray(res.results[0]["out"], dtype=np.float32)


# revision 20
# speedup vs baseline: 1.6275x; 1.0035x over previous
"""Trainium2 Bass kernel for nn_Classifier (segment_reduce).

Computation (reference semantics):
  attn  = concat(emb, pos) @ W_attn + b_attn          (S, T, 1)
  w     = softmax(attn, axis=1)                        per-segment over T
  segv  = sum_t w * emb                                (S, BERT)
  vecs  = segment_sum(segv, segment_ids, 64)           (64, BERT)
  out   = sigmoid(lrelu(lrelu(vecs@W1+b1)@W2+b2)@W3+b3)

Sharding: data-parallel over S across 8 NeuronCores (32 segments each),
AllReduce of the (768, 64) transposed comment partials, replicated MLP.

Key optimizations vs the fp32 baseline:
 - emb||pos concatenated, token-permuted to (p, i) order (softmax pooling
   is token-permutation invariant) and cast to bf16 ON HOST: halves HBM
   traffic, gives 7KB contiguous partition lines, and runs the PE at
   1 cycle/row instead of fp32's LOW_HIGH 2-pass mode.
 - AllReduce split into two chunks: segments 0:16 reduce + AllReduce
   launch mid-loop (hidden under the second half of the segment loop);
   only the second 196KB AllReduce is exposed at the end.
 - comment partials computed TRANSPOSED (feature-major), so the MLP runs
   transpose-free: h^T tiles (128 feat x 64 comments), per-partition
   bias fused into the PSUM->SBUF copy, lrelu as max(0.01x, x) on DVE.
 - dependent DMAs (collective staging, segvec scatter) issue from
   scalar/gpsimd queues so they never head-of-line block the streaming
   ep DMAs on the sync queue.

Notes:
 - b_attn shifts all logits of a segment equally -> softmax-invariant -> dropped.
 - softmax normalization (1/sum_t exp) is folded into the host-built
   one-hot segment->comment matrix, scaled on device by 1/den per segment.
 - exp() without max-subtraction: logits ~ N(0, 0.6^2), safe.
 - addr_space="Shared" collective outputs crash this runtime
   (NRT_EXEC_UNIT_UNRECOVERABLE); Local DRAM tiles work.
"""

import sys

sys.path.insert(0, "/opt/trn_rl_repo")

import numpy as np

# Full-problem dims (hardcoded per contract)
S, T, BERT, POS = 256, 512, 768, 128
FEAT = BERT + POS
H1 = 1024
NCLS = 6
NCOM = 64
NCORES = 8
NT = T // 128  # token tiles per segment
NB = BERT // 128  # feature blocks of emb (6)
NJ1 = BERT // 128  # k tiles layer1 (6)
NJ2 = H1 // 128  # k tiles layer2/3 (8)

_CACHE = {}


def build_nc(n_cores, sl):
    """Build the SPMD Bass program for one core (sl segments/core)."""
    import concourse.bass as bass
    import concourse.mybir as mybir
    import concourse.tile as tile
    from concourse import bacc

    f32 = mybir.dt.float32
    bf16 = mybir.dt.bfloat16
    AF = mybir.ActivationFunctionType
    OP = mybir.AluOpType
    AX = mybir.AxisListType

    half = sl // 2  # chunk boundary for the split AllReduce

    nc = bacc.Bacc(
        "TRN2", target_bir_lowering=False, debug=False, num_devices=n_cores
    )

    ep_d = nc.dram_tensor("ep", [sl, 128, NT * FEAT], bf16, kind="ExternalInput").ap()
    wab_d = nc.dram_tensor("wab", [128, FEAT], bf16, kind="ExternalInput").ap()
    oneh_d = nc.dram_tensor("oneh", [sl, NCOM], f32, kind="ExternalInput").ap()
    w1_d = nc.dram_tensor("w1", [128, NJ1, H1], bf16, kind="ExternalInput").ap()
    w2_d = nc.dram_tensor("w2", [128, NJ2, H1], bf16, kind="ExternalInput").ap()
    w3_d = nc.dram_tensor("w3", [128, NJ2, NCLS], bf16, kind="ExternalInput").ap()
    b1t_d = nc.dram_tensor("b1t", [128, NJ2], f32, kind="ExternalInput").ap()
    b2t_d = nc.dram_tensor("b2t", [128, NJ2], f32, kind="ExternalInput").ap()
    b3_d = nc.dram_tensor("b3", [1, NCLS], f32, kind="ExternalInput").ap()
    out_d = nc.dram_tensor("out", [NCOM, NCLS], f32, kind="ExternalOutput").ap()

    with tile.TileContext(nc) as tc:
        with (
            tc.tile_pool(name="const", bufs=1) as const_pool,
            tc.tile_pool(name="ep", bufs=10) as ep_pool,
            tc.tile_pool(name="work", bufs=1) as work,
            tc.tile_pool(name="psv", bufs=2, space="PSUM") as psv,
            tc.tile_pool(name="pcm", bufs=2, space="PSUM") as pcm,
            tc.tile_pool(name="pmlp", bufs=2, space="PSUM") as pmlp,
            tc.tile_pool(name="dram", bufs=1, space="DRAM") as dram,
        ):
            # ---- constants (small, loaded first) ----
            wab_sb = const_pool.tile([128, FEAT], bf16)
            nc.sync.dma_start(wab_sb, wab_d)
            # one-hot loaded as two half tiles so every engine read starts
            # at partition 0 (partition base must be 0/32/64)
            oneh_sbA = const_pool.tile([sl // 2, NCOM], f32)
            nc.sync.dma_start(oneh_sbA, oneh_d[0 : sl // 2])
            oneh_sbB = const_pool.tile([sl // 2, NCOM], f32)
            nc.sync.dma_start(oneh_sbB, oneh_d[sl // 2 : sl])
            ones_bf = const_pool.tile([128, 1], bf16)
            nc.gpsimd.memset(ones_bf, 1.0)
            onesf = const_pool.tile([1, NCOM], f32)
            nc.gpsimd.memset(onesf, 1.0)

            # ---- persistent working tiles ----
            L_sb = work.tile([128, NT * sl], f32)   # logits, col = 4*s + i
            E_sb = work.tile([128, NT * sl], bf16)  # exp(logits)
            # unnormalized segvecs, one tile per AllReduce chunk so each
            # matmul stationary read starts at partition 0
            segvecsA = work.tile([sl // 2, BERT], f32)
            segvecsB = work.tile([sl // 2, BERT], f32)
            # real step-1 discard tile for the fused logit reduce: a
            # stride-0 broadcast dst forces the DVE into 1x mode (the two
            # packed results would target the same word); a dense bf16 dst
            # keeps the 2x_1p write path available.
            dummy = work.tile([128, FEAT], bf16)

            def chunk_reduce(lo, hi, segv, oneh_half, tag):
                """den + scaled one-hot + transposed comment partial matmul +
                AllReduce for local segments [lo, hi). Returns the SBUF tile
                that will hold the reduced (128, NB*NCOM) partial."""
                n = hi - lo
                den_ps = pcm.tile([1, NT * n], f32, tag="cm")
                nc.tensor.matmul(
                    den_ps, ones_bf, E_sb[:, NT * lo : NT * hi], start=True, stop=True
                )
                den_sb = work.tile([1, n], f32, tag=f"den{tag}", name=f"den{tag}")
                nc.vector.tensor_reduce(
                    den_sb,
                    den_ps.rearrange("p (s i) -> p s i", i=NT),
                    axis=AX.X,
                    op=OP.add,
                )
                denT = pcm.tile([n, 1], f32, tag="cm")
                nc.tensor.matmul(denT, den_sb, onesf[0:1, 0:1], start=True, stop=True)
                inv_den = work.tile([n, 1], f32, tag=f"inv{tag}", name=f"inv{tag}")
                nc.vector.reciprocal(inv_den, denT)
                oneh_sc = work.tile([n, NCOM], f32, tag=f"ohs{tag}", name=f"ohs{tag}")
                nc.vector.tensor_scalar_mul(oneh_sc, oneh_half, inv_den)
                # cmT[b*128+p, c] = sum_s segvecs[s, b*128+p] * oneh_sc[s, c]
                cmT_ps = pcm.tile([128, NB * NCOM], f32, tag="cm")
                for b in range(NB):
                    nc.tensor.matmul(
                        cmT_ps[:, b * NCOM : (b + 1) * NCOM],
                        segv[0:n, 128 * b : 128 * (b + 1)],
                        oneh_sc,
                        start=True,
                        stop=True,
                    )
                # bf16 collective payload: halves the ring transfer; the
                # partials are O(1) magnitudes so 0.4% rel err is fine.
                cmT_sb = work.tile(
                    [128, NB * NCOM], bf16, tag=f"cmT{tag}", name=f"cmT{tag}"
                )
                nc.vector.tensor_copy(cmT_sb, cmT_ps)
                ar_in = dram.tile([128, NB * NCOM], bf16, tag=f"ari{tag}")
                ar_out = dram.tile([128, NB * NCOM], bf16, tag=f"aro{tag}")
                nc.gpsimd.dma_start(ar_in, cmT_sb)
                nc.gpsimd.collective_compute(
                    "AllReduce",
                    OP.add,
                    replica_groups=[list(range(n_cores))],
                    ins=[ar_in.opt()],
                    outs=[ar_out.opt()],
                )
                vecs_sb = work.tile(
                    [128, NB * NCOM], bf16, tag=f"vec{tag}", name=f"vec{tag}"
                )
                nc.gpsimd.dma_start(vecs_sb, ar_out)
                return vecs_sb

            # ---- main loop over local segments ----
            vecsA = None
            sv = None
            for s in range(sl):
                ep = ep_pool.tile([128, NT * FEAT], bf16, tag="ep")
                # Split each segment's load across DMA engines; the first
                # few segments split 4-ways to cut pipeline-fill latency.
                nd = 4 if s < 4 else 2
                step = (NT * FEAT) // nd
                for k in range(nd):
                    nc.sync.dma_start(
                        ep[:, k * step : (k + 1) * step],
                        ep_d[s, :, k * step : (k + 1) * step],
                    )
                # attention logits: fused multiply + free-dim reduce
                for i in range(NT):
                    col = NT * s + i
                    nc.vector.scalar_tensor_tensor(
                        dummy,
                        ep[:, i * FEAT : (i + 1) * FEAT],
                        1.0,
                        wab_sb,
                        op0=OP.mult,
                        op1=OP.mult,
                        accum_out=L_sb[:, col : col + 1],
                    )
                # e = exp(logits)
                nc.scalar.activation(
                    E_sb[:, NT * s : NT * s + NT],
                    L_sb[:, NT * s : NT * s + NT],
                    AF.Exp,
                )
                # pooling: segvec[s] = sum_t e[t] * emb[t, :]
                sv = psv.tile([1, BERT], f32, tag="sv")
                for i in range(NT):
                    col = NT * s + i
                    for n0, n1 in ((0, 512), (512, BERT)):
                        nc.tensor.matmul(
                            sv[0:1, n0:n1],
                            E_sb[:, col : col + 1],
                            ep[:, i * FEAT + n0 : i * FEAT + n1],
                            start=(i == 0),
                            stop=(i == NT - 1),
                        )
                stage = work.tile([1, BERT], f32, tag="stage", bufs=4, name="stage")
                nc.scalar.copy(stage, sv)
                # partition-scatter into row s of the half's segvec tile;
                # issued from the scalar queue right after its producing copy.
                segv = segvecsA if s < half else segvecsB
                row = s % half
                nc.scalar.dma_start(segv[row : row + 1, :], stage)

                if s == 2:
                    # MLP weights: emitted here so the sync queue issues
                    # them early; ~3.7MB bf16 trickles in under the loop.
                    b1t_sb = const_pool.tile([128, NJ2], f32)
                    nc.sync.dma_start(b1t_sb, b1t_d)
                    b2t_sb = const_pool.tile([128, NJ2], f32)
                    nc.sync.dma_start(b2t_sb, b2t_d)
                    b3_sb = const_pool.tile([1, NCLS], f32)
                    nc.sync.dma_start(b3_sb, b3_d)
                    w1_sb = const_pool.tile([128, NJ1, H1], bf16)
                    nc.sync.dma_start(w1_sb, w1_d)
                    w2_sb = const_pool.tile([128, NJ2, H1], bf16)
                    nc.sync.dma_start(w2_sb, w2_d)
                    w3_sb = const_pool.tile([128, NJ2, NCLS], bf16)
                    nc.sync.dma_start(w3_sb, w3_d)

                if s == half - 1:
                    # first-chunk comment partials + AllReduce, hidden
                    # under the second half of the segment loop
                    vecsA = chunk_reduce(0, half, segvecsA, oneh_sbA, "A")

            vecsB = chunk_reduce(half, sl, segvecsB, oneh_sbB, "B")

            # vecsT = vecsA + vecsB (both bf16 already)
            vecsT = work.tile([128, NB * NCOM], bf16)
            nc.vector.scalar_tensor_tensor(
                vecsT, vecsA, 1.0, vecsB, op0=OP.mult, op1=OP.add
            )

            # ---- MLP, transpose-free (feature-major activations) ----
            def layer_T(x_bf, nj, nout, w_sb, bT_sb, lid):
                """x_bf: (128, nj*NCOM) bf16 -> returns (128, nout*NCOM) bf16
                with hT[n*128+p, c] = lrelu(sum_jp W[jp, n*128+p] x[jp, c] + b)."""
                y = work.tile([128, nout * NCOM], bf16, tag=f"y{lid}", name="yT")
                for n in range(nout):
                    hps = pmlp.tile([128, NCOM], f32, tag="h")
                    for j in range(nj):
                        nc.tensor.matmul(
                            hps,
                            w_sb[:, j, 128 * n : 128 * (n + 1)],
                            x_bf[:, j * NCOM : (j + 1) * NCOM],
                            start=(j == 0),
                            stop=(j == nj - 1),
                        )
                    xs = work.tile([128, NCOM], f32, tag=f"xs{lid}", bufs=3, name="xs")
                    # PSUM->SBUF copy with fused per-partition bias add
                    nc.scalar.activation(
                        xs, hps, AF.Identity, bias=bT_sb[:, n : n + 1]
                    )
                    # lrelu: y = max(0.01*x, x)
                    nc.vector.scalar_tensor_tensor(
                        y[:, n * NCOM : (n + 1) * NCOM],
                        xs,
                        0.01,
                        xs,
                        op0=OP.mult,
                        op1=OP.max,
                    )
                return y

            h1T = layer_T(vecsT, NJ1, NJ2, w1_sb, b1t_sb, 1)
            h2T = layer_T(h1T, NJ2, NJ2, w2_sb, b2t_sb, 2)

            # layer 3: out (NCOM, NCLS) = sum_j h2T_j^T @ W3_j + b3
            ops = pmlp.tile([NCOM, NCLS], f32, tag="h")
            for j in range(NJ2):
                nc.tensor.matmul(
                    ops,
                    h2T[:, j * NCOM : (j + 1) * NCOM],
                    w3_sb[:, j, :],
                    start=(j == 0),
                    stop=False,
                )
            nc.tensor.matmul(ops, onesf, b3_sb, start=False, stop=True)
            out_sb = work.tile([NCOM, NCLS], f32)
            nc.scalar.activation(out_sb, ops, AF.Sigmoid)
            nc.sync.dma_start(out_d, out_sb)

    nc.compile()
    return nc


def make_in_maps(
    embeddings,
    position_encodings,
    W_attn,
    W1,
    b1,
    W2,
    b2,
    W3,
    b3,
    segment_ids,
    n_cores,
    ncom,
):
    """Host-side sharding: slice S across cores, build per-core one-hot,
    concat+permute+bf16-cast the token stream, transpose weights."""
    import ml_dtypes

    f32 = np.float32
    bf16 = ml_dtypes.bfloat16
    s_total = embeddings.shape[0]
    sl = s_total // n_cores

    emb = np.asarray(embeddings, dtype=f32)
    pos = np.asarray(position_encodings, dtype=f32)
    epc = np.concatenate([emb, pos], axis=2)  # (S, T, FEAT)
    # token t = i*128 + p  ->  (p, i); softmax pooling is token-perm invariant
    epc = epc.reshape(s_total, NT, 128, FEAT).transpose(0, 2, 1, 3)
    epc = np.ascontiguousarray(epc).reshape(s_total, 128, NT * FEAT).astype(bf16)

    wa = np.asarray(W_attn, dtype=f32).reshape(-1)
    wab = np.ascontiguousarray(np.tile(wa[None, :], (128, 1))).astype(bf16)

    w1r = np.ascontiguousarray(
        np.asarray(W1, dtype=f32).reshape(NJ1, 128, H1).transpose(1, 0, 2)
    ).astype(bf16)
    w2r = np.ascontiguousarray(
        np.asarray(W2, dtype=f32).reshape(NJ2, 128, H1).transpose(1, 0, 2)
    ).astype(bf16)
    w3r = np.ascontiguousarray(
        np.asarray(W3, dtype=f32).reshape(NJ2, 128, NCLS).transpose(1, 0, 2)
    ).astype(bf16)
    b1t = np.ascontiguousarray(np.asarray(b1, dtype=f32).reshape(NJ2, 128).T)
    b2t = np.ascontiguousarray(np.asarray(b2, dtype=f32).reshape(NJ2, 128).T)
    b3r = np.ascontiguousarray(np.asarray(b3, dtype=f32).reshape(1, -1))

    seg = np.asarray(segment_ids).astype(np.int64).reshape(-1)
    common = {
        "wab": wab,
        "w1": w1r,
        "w2": w2r,
        "w3": w3r,
        "b1t": b1t,
        "b2t": b2t,
        "b3": b3r,
    }
    in_maps = []
    for c in range(n_cores):
        oneh = np.zeros((sl, ncom), dtype=f32)
        local = seg[c * sl : (c + 1) * sl]
        oneh[np.arange(sl), local] = 1.0
        in_maps.append(
            {
                "ep": np.ascontiguousarray(epc[c * sl : (c + 1) * sl]),
                "oneh": oneh,
                **common,
            }
        )
    return in_maps


def kernel(
    embeddings,
    position_encodings,
    W_attn,
    b_attn,
    W1,
    b1,
    W2,
    b2,
    W3,
    b3,
    segment_ids,
    num_comments,
):
    from concourse.bass_utils import run_bass_kernel_spmd

    assert int(num_comments) == NCOM
    assert embeddings.shape == (S, T, BERT)
    assert position_encodings.shape == (S, T, POS)
    # b_attn shifts every logit of a segment equally -> softmax-invariant.

    key = "full"
    if key not in _CACHE:
        _CACHE[key] = build_nc(NCORES, S // NCORES)
    nc = _CACHE[key]

    in_maps = make_in_maps(
        embeddings,
        position_encodings,
        W_attn,
        W1,
        b1,
        W2,
        b2,
        W3,
        b3,
        segment_ids,
        NCORES,
        NCOM,
    )
    res = run_bass_kernel_spmd(nc, in_maps, list(range(NCORES)))
    return np.asarray(res.results[0]["out"], dtype=np.float32)
